# revision 3
# baseline (speedup 1.0000x reference)
"""Chamfer distance v10: block-sparse KNN structure.

Host sorts the points per batch (data layout prep). Consecutive runs of 128
sorted points form blocks; for each block only a tiny contiguous run of
sorted bin-centers (L<=8 wide for this data) can contain any point's nearest
center, or have its own nearest point inside the block. Device computes, per
block, the negated scaled distance grid [128 points x 8 candidate slots] via
one tiny PE matmul (exact f32: G = -(x^2) + x*(2y) + (-1)*(y^2), x=S(t-a),
y=S(c-a), a = per-block origin), then:
  dir2 (per-point min over centers): DVE max-tree over the 8 slots, then
        ScalarE accum-sum per batch.
  dir1 (per-center min over points): gpsimd partition_all_reduce(max) over
        the 128 point-partitions; host scatter-mins the per-block rows.
All mins/sums are computed on device; the host does sorting, metadata
(searchsorted of block bounds), and the final tiny combines.
"""

import sys

if "/opt/trn_rl_repo" not in sys.path:
    sys.path.insert(0, "/opt/trn_rl_repo")

import numpy as np

import concourse.bass as bass
import concourse.tile as tile
from concourse import bacc, mybir, bass_isa
from concourse.bass_utils import run_bass_kernel_spmd

B = 2
N = 76800
E = 257
K = 256
NCORES = 8
BLK = 128
NBLK_B = N // BLK          # 600 blocks per batch
BPB = NBLK_B // NCORES     # 75 blocks per (core, batch)
BPC = B * BPB              # 150 blocks per core
L = 8                      # candidate-center slots per block (data max is 6)
S = 1024.0
FAR = 1.0e9

F32 = mybir.dt.float32
F16 = mybir.dt.float16
MAX = mybir.AluOpType.max
COPY = mybir.ActivationFunctionType.Copy

STAT_DMA_SPLIT = 5         # DMA chunks per half for the stationary load


def _build_kernel(nc, tc, stat_in, mov_in, d1_out, d2_out):
    from contextlib import ExitStack

    ctx = ExitStack()
    sb = ctx.enter_context(tc.tile_pool(name="sb", bufs=1))
    psum_pool = ctx.enter_context(tc.tile_pool(name="ps", bufs=1, space="PSUM"))

    stat_sb = sb.tile([3, BPC, BLK], F32, tag="stat")
    mov_sb = sb.tile([3, BPC, L], F32, tag="mov")
    ps = psum_pool.tile([128, B, BPB, L], F32, tag="grid")
    gsb = sb.tile([128, B, BPB, L], F16, tag="gsb")
    l1 = sb.tile([128, B, BPB, 4], F16, tag="l1")
    l2 = sb.tile([128, B, BPB, 2], F16, tag="l2")
    l3 = sb.tile([128, B, BPB, 1], F16, tag="l3")
    junk = sb.tile([128, BPB], F16, tag="junk")
    acc = sb.tile([128, B], F32, tag="acc")
    parout = sb.tile([128, B, BPB, L], F16, tag="parout")

    nc.sync.dma_start(mov_sb[:], mov_in)

    csz = BPB // STAT_DMA_SPLIT  # 15 blocks per stat chunk
    for h in range(B):
        for s in range(STAT_DMA_SPLIT):
            lo = h * BPB + s * csz
            eng = nc.sync if (s % 2 == 0) else nc.scalar
            eng.dma_start(stat_sb[:, lo : lo + csz], stat_in[:, lo : lo + csz])

    for h in range(B):
        for i in range(BPB):
            blk = h * BPB + i
            nc.tensor.matmul(
                ps[:, h, i, :],
                stat_sb[:, blk, :],
                mov_sb[:, blk, :],
                start=True,
                stop=True,
            )
        # single PSUM->SBUF f16 squash, then all reductions in f16 SBUF
        nc.scalar.activation(gsb[:, h], ps[:, h], COPY)
        nc.vector.tensor_tensor(
            l1[:, h], gsb[:, h, :, 0:4], gsb[:, h, :, 4:8], op=MAX
        )
        nc.vector.tensor_tensor(
            l2[:, h], l1[:, h, :, 0:2], l1[:, h, :, 2:4], op=MAX
        )
        nc.vector.tensor_tensor(
            l3[:, h], l2[:, h, :, 0:1], l2[:, h, :, 1:2], op=MAX
        )
        nc.scalar.activation(
            junk[:], l3[:, h, :, 0], COPY, accum_out=acc[:, h : h + 1]
        )
        nc.gpsimd.partition_all_reduce(
            parout[:, h], gsb[:, h], channels=128, reduce_op=bass_isa.ReduceOp.max
        )
        nc.sync.dma_start(d1_out[h], parout[0:1, h])

    nc.sync.dma_start(d2_out, acc[:])
    ctx.close()


_CACHE = {}


def _get_compiled():
    if "nc" in _CACHE:
        return _CACHE["nc"]
    nc = bacc.Bacc(
        "TRN2",
        target_bir_lowering=False,
        debug=False,
        enable_asserts=False,
        num_devices=NCORES,
    )
    stat_in = nc.dram_tensor("stat", [3, BPC, BLK], F32, kind="ExternalInput").ap()
    mov_in = nc.dram_tensor("mov", [3, BPC, L], F32, kind="ExternalInput").ap()
    d1_out = nc.dram_tensor("d1", [B, BPB, L], F16, kind="ExternalOutput").ap()
    d2_out = nc.dram_tensor("d2", [128, B], F32, kind="ExternalOutput").ap()

    with tile.TileContext(nc) as tc:
        _build_kernel(nc, tc, stat_in, mov_in, d1_out, d2_out)
    nc.compile()
    _CACHE["nc"] = nc
    return nc


def _prep(target: np.ndarray, bin_edges: np.ndarray):
    """Host prep: sort, block metadata, limb/shift arrays. Returns
    (in_maps, meta) where meta holds per-core (b, lo, L) block info."""
    target = np.asarray(target, dtype=np.float32).reshape(B, N)
    edges = np.asarray(bin_edges, dtype=np.float64)

    in_maps = [dict() for _ in range(NCORES)]
    meta = []
    for c in range(NCORES):
        meta.append({"lo": np.zeros((B, BPB), np.int64),
                     "ln": np.zeros((B, BPB), np.int64)})

    stat_all = np.zeros((NCORES, 3, BPC, BLK), np.float32)
    mov_all = np.zeros((NCORES, 3, BPC, L), np.float32)
    cts_sorted = []

    for b in range(B):
        pts = np.sort(target[b])
        cts = np.sort(0.5 * (edges[b, :-1] + edges[b, 1:]))
        cts_sorted.append(cts)
        pts64 = pts.astype(np.float64)

        t0s = pts64[0::BLK]            # [600]
        t1s = pts64[BLK - 1 :: BLK]
        tprev = np.concatenate(([-np.inf], t1s[:-1]))
        tnext = np.concatenate((t0s[1:], [np.inf]))

        lo = np.minimum(
            np.searchsorted(cts, tprev, side="right"),
            np.searchsorted(cts, t0s, side="right") - 1,
        )
        lo = np.maximum(lo, 0)
        hi = np.maximum(
            np.searchsorted(cts, tnext, side="left") - 1,
            np.searchsorted(cts, t1s, side="left"),
        )
        hi = np.minimum(hi, K - 1)
        ln = hi - lo + 1
        assert ln.max() <= L, f"candidate run {ln.max()} exceeds L={L}"

        a = t0s                                        # [600] block origins
        x = S * (pts64.reshape(NBLK_B, BLK) - a[:, None])   # [600, 128]
        x32 = x.astype(np.float32)
        row0 = (-(x32.astype(np.float64) ** 2)).astype(np.float32)

        # moving: per block, per slot j: (1, 2y, y^2) or (1, 0, FAR) pad
        idx = lo[:, None] + np.arange(L)[None, :]      # [600, L]
        valid = np.arange(L)[None, :] < ln[:, None]
        idxc = np.clip(idx, 0, K - 1)
        y = S * (cts[idxc] - a[:, None])
        y32 = y.astype(np.float32)
        m1 = np.where(valid, 2.0 * y32.astype(np.float64), 0.0).astype(np.float32)
        m2 = np.where(valid, y32.astype(np.float64) ** 2, FAR).astype(np.float32)

        for c in range(NCORES):
            sl = slice(c * BPB, (c + 1) * BPB)
            dst = slice(b * BPB, (b + 1) * BPB)
            stat_all[c, 0, dst] = row0[sl]
            stat_all[c, 1, dst] = x32[sl]
            stat_all[c, 2, dst] = -1.0
            mov_all[c, 0, dst] = 1.0
            mov_all[c, 1, dst] = m1[sl]
            mov_all[c, 2, dst] = m2[sl]
            meta[c]["lo"][b] = lo[sl]
            meta[c]["ln"][b] = ln[sl]

    for c in range(NCORES):
        in_maps[c] = {
            "stat": np.ascontiguousarray(stat_all[c]),
            "mov": np.ascontiguousarray(mov_all[c]),
        }
    return in_maps, meta, cts_sorted


def _combine(results, meta):
    d2_tot = np.zeros(B, np.float64)
    gmax = np.full((B, K), -np.inf)
    for c, res in enumerate(results):
        d2 = np.asarray(res["d2"], np.float64)          # [128, B]
        d2_tot += d2.sum(axis=0)
        d1 = np.asarray(res["d1"], np.float64)          # [B, BPB, L]
        lo, ln = meta[c]["lo"], meta[c]["ln"]
        for b in range(B):
            for i in range(BPB):
                li, ll = lo[b, i], ln[b, i]
                row = d1[b, i, :ll]
                np.maximum.at(gmax[b], np.arange(li, li + ll), row)
    assert np.isfinite(gmax).all(), "uncovered center in dir1 combine"
    dir2 = -d2_tot / (S * S)
    dir1 = (-gmax / (S * S)).sum(axis=1)
    return np.float32((dir1 + dir2).mean())


def kernel(target: np.ndarray, bin_edges: np.ndarray) -> np.ndarray:
    in_maps, meta, _ = _prep(target, bin_edges)
    nc = _get_compiled()
    res = run_bass_kernel_spmd(nc, in_maps, list(range(NCORES))).results
    out = _combine(res, meta)
    return np.asarray(out, dtype=np.float32)


# revision 11
# speedup vs baseline: 2.4994x; 2.4994x over previous
"""Chamfer distance v11: block-sparse KNN + block-diagonal grouped matmuls.

Host sorts points per batch; 128-point sorted blocks each have a contiguous
run of <=8 candidate sorted centers (data max 6). 16 blocks are packed into
one fp32 matmul via a block-diagonal stationary; slot j of block gb lands at
output partition 16*j+gb (matches the DMA-transpose layout). Grid value:
    G = -(x^2)*1 + x*(2y) + bias(-y^2),  x = S(t - a_blk), y = S(c - a_blk)
K = 2 rows per block * 16 blocks = 32; the -y^2 term is applied as the
per-partition bias of the ScalarE squash (Identity activation), so it needs
no matmul rows. One LdWeights per 16 blocks instead of one per block (the
v10 killer: 300 LdWeights = 64us).

Reductions per group g (PSUM [128 blockslots, 128 points]):
  squash: ScalarE Identity+bias -> f16 SBUF
  dir1 (per-center min over points): free-axis max-folds (Pool+DVE) -> [128, NG]
  dir2 (per-point min over centers): DMA-transpose [128,128], DVE max-tree over
        own block's 8 slots -> per-point maxes, ScalarE accum-sum per batch.
Host: sorting, block metadata, final tiny combines (exact, f64).
"""

import sys

if "/opt/trn_rl_repo" not in sys.path:
    sys.path.insert(0, "/opt/trn_rl_repo")

import numpy as np

import concourse.bass as bass
import concourse.tile as tile
from concourse import bacc, mybir
from concourse.bass_utils import run_bass_kernel_spmd

B = 2
N = 76800
E = 257
K = 256
NCORES = 8
BLK = 128
NBLK_B = N // BLK          # 600 blocks per batch
BPB = NBLK_B // NCORES     # 75 blocks per (core, batch)
GBLK = 16                  # blocks per matmul group
NGH = 5                    # groups per batch-half (5*16 = 80 slots, 75 real)
NG = B * NGH               # 10 groups per core
L = 8                      # candidate-center slots per block (data max is 6)
KK = 2 * GBLK              # contraction rows per group matmul (s row + x row)
S = 1024.0
FAR = 3.0e4

F32 = mybir.dt.float32
F16 = mybir.dt.float16
MAX = mybir.AluOpType.max
COPY = mybir.ActivationFunctionType.Copy
IDENT = mybir.ActivationFunctionType.Identity
AX = mybir.AxisListType

DMA_GRP = 2                # groups per input DMA chunk


def _build_kernel(nc, tc, w_in, m_in, qb_in, d1_out, d2_out):
    from contextlib import ExitStack

    ctx = ExitStack()
    sb = ctx.enter_context(tc.tile_pool(name="sb", bufs=1))
    psum_pool = ctx.enter_context(tc.tile_pool(name="ps", bufs=1, space="PSUM"))

    w_sb = sb.tile([KK, NG, 128], F32, tag="w")
    m_sb = sb.tile([KK, NG, 128], F32, tag="m")
    qb_sb = sb.tile([128, NG], F32, tag="qb")
    ps = psum_pool.tile([128, NG, 128], F32, tag="grid")
    gsb = sb.tile([128, NG, 128], F16, tag="gsb")
    tt = sb.tile([128, NG, GBLK, L], F16, tag="tt")
    # dir2 tree
    l1 = sb.tile([128, B, NGH, GBLK, 4], F16, tag="l1")
    l2 = sb.tile([128, B, NGH, GBLK, 2], F16, tag="l2")
    l3 = sb.tile([128, B, NGH, GBLK, 1], F16, tag="l3")
    junk = sb.tile([128, NGH * GBLK], F16, tag="junk")
    acc = sb.tile([128, B], F32, tag="acc")
    # dir1 folds
    f1 = sb.tile([128, B, NGH, 64], F16, tag="f1")
    f2 = sb.tile([128, B, NGH, 32], F16, tag="f2")
    f3 = sb.tile([128, B, NGH, 16], F16, tag="f3")
    f4 = sb.tile([128, B, NGH, 8], F16, tag="f4")
    d1sb = sb.tile([128, B, NGH, 1], F16, tag="d1sb")

    nc.sync.dma_start(qb_sb[:], qb_in)
    for g0 in range(0, NG, DMA_GRP):
        gs = slice(g0, g0 + DMA_GRP)
        eng = nc.sync if (g0 // DMA_GRP) % 2 == 0 else nc.scalar
        eng.dma_start(w_sb[:, gs], w_in[:, gs])
        eng.dma_start(m_sb[:, gs], m_in[:, gs])

    for g in range(NG):
        nc.tensor.matmul(
            ps[:, g], w_sb[:, g], m_sb[:, g], start=True, stop=True
        )
        nc.scalar.activation(
            gsb[:, g], ps[:, g], IDENT, bias=qb_sb[:, g : g + 1]
        )
        eng = nc.sync if (g % 2 == 0) else nc.scalar
        eng.dma_start_transpose(tt[:, g], gsb[:, g])

    for h in range(B):
        hs = slice(h * NGH, (h + 1) * NGH)
        # dir1: fold points (free axis of gsb) 128 -> 1
        nc.vector.tensor_tensor(
            f1[:, h], gsb[:, hs, 0:64], gsb[:, hs, 64:128], op=MAX
        )
        nc.vector.tensor_tensor(
            f2[:, h], f1[:, h, :, 0:32], f1[:, h, :, 32:64], op=MAX
        )
        nc.vector.tensor_tensor(
            f3[:, h], f2[:, h, :, 0:16], f2[:, h, :, 16:32], op=MAX
        )
        nc.vector.tensor_tensor(
            f4[:, h], f3[:, h, :, 0:8], f3[:, h, :, 8:16], op=MAX
        )
        nc.vector.tensor_reduce(
            out=d1sb[:, h], in_=f4[:, h], op=MAX, axis=AX.X
        )
        # dir2: fold own-block slots (last axis of tt) 8 -> 1
        nc.vector.tensor_tensor(
            l1[:, h], tt[:, hs, :, 0:4], tt[:, hs, :, 4:8], op=MAX
        )
        nc.vector.tensor_tensor(
            l2[:, h], l1[:, h, :, :, 0:2], l1[:, h, :, :, 2:4], op=MAX
        )
        nc.vector.tensor_tensor(
            l3[:, h], l2[:, h, :, :, 0:1], l2[:, h, :, :, 1:2], op=MAX
        )
        nc.scalar.activation(
            junk[:], l3[:, h, :, :, 0], COPY, accum_out=acc[:, h : h + 1]
        )

    nc.sync.dma_start(d1_out, d1sb[:])
    nc.sync.dma_start(d2_out, acc[:])
    ctx.close()


_CACHE = {}


def _get_compiled():
    if "nc" in _CACHE:
        return _CACHE["nc"]
    nc = bacc.Bacc(
        "TRN2",
        target_bir_lowering=False,
        debug=False,
        enable_asserts=False,
        num_devices=NCORES,
    )
    w_in = nc.dram_tensor("w", [KK, NG, 128], F32, kind="ExternalInput").ap()
    m_in = nc.dram_tensor("m", [KK, NG, 128], F32, kind="ExternalInput").ap()
    qb_in = nc.dram_tensor("qb", [128, NG], F32, kind="ExternalInput").ap()
    d1_out = nc.dram_tensor("d1", [128, B, NGH, 1], F16, kind="ExternalOutput").ap()
    d2_out = nc.dram_tensor("d2", [128, B], F32, kind="ExternalOutput").ap()

    with tile.TileContext(nc) as tc:
        _build_kernel(nc, tc, w_in, m_in, qb_in, d1_out, d2_out)
    nc.compile()
    _CACHE["nc"] = nc
    return nc


def _prep(target: np.ndarray, bin_edges: np.ndarray):
    """Host prep: sort, block metadata, packed W/M/bias arrays.

    Returns (in_maps, meta, cts_sorted); meta[c]["lo"/"ln"] are [NG, GBLK]
    (ln == 0 marks a padding block slot)."""
    target = np.asarray(target, dtype=np.float32).reshape(B, N)
    edges = np.asarray(bin_edges, dtype=np.float64)

    w_all = np.zeros((NCORES, KK, NG, 128), np.float32)
    m_all = np.zeros((NCORES, KK, NG, 128), np.float32)
    qb_all = np.zeros((NCORES, 128, NG), np.float32)
    meta = [
        {"lo": np.zeros((NG, GBLK), np.int64), "ln": np.zeros((NG, GBLK), np.int64)}
        for _ in range(NCORES)
    ]
    cts_sorted = []

    for b in range(B):
        pts = np.sort(target[b])
        cts = np.sort(0.5 * (edges[b, :-1] + edges[b, 1:]))
        cts_sorted.append(cts)
        pts64 = pts.astype(np.float64)

        t0s = pts64[0::BLK]
        t1s = pts64[BLK - 1 :: BLK]
        tprev = np.concatenate(([-np.inf], t1s[:-1]))
        tnext = np.concatenate((t0s[1:], [np.inf]))

        lo = np.minimum(
            np.searchsorted(cts, tprev, side="right"),
            np.searchsorted(cts, t0s, side="right") - 1,
        )
        lo = np.maximum(lo, 0)
        hi = np.maximum(
            np.searchsorted(cts, tnext, side="left") - 1,
            np.searchsorted(cts, t1s, side="left"),
        )
        hi = np.minimum(hi, K - 1)
        ln = hi - lo + 1
        assert ln.max() <= L, f"candidate run {ln.max()} exceeds L={L}"

        a = t0s
        x = (S * (pts64.reshape(NBLK_B, BLK) - a[:, None])).astype(np.float32)
        row0 = (-(x.astype(np.float64) ** 2)).astype(np.float32)   # [600, 128]

        idx = lo[:, None] + np.arange(L)[None, :]
        valid = np.arange(L)[None, :] < ln[:, None]
        idxc = np.clip(idx, 0, K - 1)
        y = (S * (cts[idxc] - a[:, None])).astype(np.float32)      # [600, L]
        wy = np.where(valid, 2.0 * y.astype(np.float64), 0.0).astype(np.float32)
        wq = np.where(valid, -(y.astype(np.float64) ** 2), -FAR).astype(np.float32)

        for c in range(NCORES):
            blks = np.arange(c * BPB, (c + 1) * BPB)
            for s_i, gblk in enumerate(blks):
                g = b * NGH + s_i // GBLK
                gb = s_i % GBLK
                m_all[c, 2 * gb + 0, g, :] = row0[gblk]
                m_all[c, 2 * gb + 1, g, :] = x[gblk]
                # slot j of block gb at output partition 8*gb + j; the HW
                # DMA transpose is a plain transpose (tt[p, f] = gsb[f, p]),
                # so this groups a block's slots contiguously along the last
                # axis of tt. (CoreSim models a different XBAR permutation —
                # HW behavior probed and confirmed in probe.py.)
                cols = slice(L * gb, L * gb + L)
                w_all[c, 2 * gb + 0, g, cols] = 1.0
                w_all[c, 2 * gb + 1, g, cols] = wy[gblk]
                qb_all[c, cols, g] = wq[gblk]
                meta[c]["lo"][g, gb] = lo[gblk]
                meta[c]["ln"][g, gb] = ln[gblk]

    in_maps = [
        {
            "w": np.ascontiguousarray(w_all[c]),
            "m": np.ascontiguousarray(m_all[c]),
            "qb": np.ascontiguousarray(qb_all[c]),
        }
        for c in range(NCORES)
    ]
    return in_maps, meta, cts_sorted


def _combine(results, meta):
    d2_tot = np.zeros(B, np.float64)
    gmax = np.full((B, K), -np.inf)
    for c, res in enumerate(results):
        d2 = np.asarray(res["d2"], np.float64)               # [128, B]
        d2_tot += d2.sum(axis=0)
        d1 = np.asarray(res["d1"], np.float64)               # [128, B, NGH, 1]
        lo, ln = meta[c]["lo"], meta[c]["ln"]
        for g in range(NG):
            h, gh = divmod(g, NGH)
            for gb in range(GBLK):
                ll = ln[g, gb]
                if ll == 0:
                    continue
                li = lo[g, gb]
                vals = d1[L * gb : L * gb + ll, h, gh, 0]
                np.maximum.at(gmax[h], np.arange(li, li + ll), vals)
    assert np.isfinite(gmax).all(), "uncovered center in dir1 combine"
    dir2 = -d2_tot / (S * S)
    dir1 = (-gmax / (S * S)).sum(axis=1)
    return np.float32((dir1 + dir2).mean())


def kernel(target: np.ndarray, bin_edges: np.ndarray) -> np.ndarray:
    in_maps, meta, _ = _prep(target, bin_edges)
    nc = _get_compiled()
    res = run_bass_kernel_spmd(nc, in_maps, list(range(NCORES))).results
    out = _combine(res, meta)
    return np.asarray(out, dtype=np.float32)


# revision 15
# speedup vs baseline: 2.5397x; 1.0161x over previous
"""Chamfer distance v11: block-sparse KNN + block-diagonal grouped matmuls.

Host sorts points per batch; 128-point sorted blocks each have a contiguous
run of <=8 candidate sorted centers (data max 6). 16 blocks are packed into
one fp32 matmul via a block-diagonal stationary; slot j of block gb lands at
output partition 16*j+gb (matches the DMA-transpose layout). Grid value:
    G = -(x^2)*1 + x*(2y) + bias(-y^2),  x = S(t - a_blk), y = S(c - a_blk)
K = 2 rows per block * 16 blocks = 32; the -y^2 term is applied as the
per-partition bias of the ScalarE squash (Identity activation), so it needs
no matmul rows. One LdWeights per 16 blocks instead of one per block (the
v10 killer: 300 LdWeights = 64us).

Reductions per group g (PSUM [128 blockslots, 128 points]):
  squash: ScalarE Identity+bias -> f16 SBUF
  dir1 (per-center min over points): free-axis max-folds (Pool+DVE) -> [128, NG]
  dir2 (per-point min over centers): DMA-transpose [128,128], DVE max-tree over
        own block's 8 slots -> per-point maxes, ScalarE accum-sum per batch.
Host: sorting, block metadata, final tiny combines (exact, f64).
"""

import sys

if "/opt/trn_rl_repo" not in sys.path:
    sys.path.insert(0, "/opt/trn_rl_repo")

import numpy as np

import concourse.bass as bass
import concourse.tile as tile
from concourse import bacc, mybir
from concourse.bass_utils import run_bass_kernel_spmd

B = 2
N = 76800
E = 257
K = 256
NCORES = 8
BLK = 128
NBLK_B = N // BLK          # 600 blocks per batch
BPB = NBLK_B // NCORES     # 75 blocks per (core, batch)
GBLK = 16                  # blocks per matmul group
NGH = 5                    # groups per batch-half (5*16 = 80 slots, 75 real)
NG = B * NGH               # 10 groups per core
L = 8                      # candidate-center slots per block (data max is 6)
KK = 2 * GBLK              # contraction rows per group matmul (s row + x row)
S = 1024.0
FAR = 3.0e4

F32 = mybir.dt.float32
F16 = mybir.dt.float16
MAX = mybir.AluOpType.max
COPY = mybir.ActivationFunctionType.Copy
IDENT = mybir.ActivationFunctionType.Identity
AX = mybir.AxisListType

DMA_GRP = 2                # groups per input DMA chunk


def _build_kernel(nc, tc, w_in, m_in, qb_in, d1_out, d2_out):
    from contextlib import ExitStack

    ctx = ExitStack()
    sb = ctx.enter_context(tc.tile_pool(name="sb", bufs=1))
    psum_pool = ctx.enter_context(tc.tile_pool(name="ps", bufs=1, space="PSUM"))

    w_sb = sb.tile([KK, NG, 128], F32, tag="w")
    m_sb = sb.tile([KK, NG, 128], F32, tag="m")
    qb_sb = sb.tile([128, NG], F32, tag="qb")
    ps = psum_pool.tile([128, NG, 128], F32, tag="grid")
    gsb = sb.tile([128, NG, 128], F16, tag="gsb")
    tt = sb.tile([128, NG, 128], F16, tag="tt")
    # dir2 tree
    l1 = sb.tile([128, NG, GBLK, 4], F16, tag="l1")
    l2 = sb.tile([128, NG, GBLK, 2], F16, tag="l2")
    l3 = sb.tile([128, NG, GBLK, 1], F16, tag="l3")
    junk = sb.tile([128, NGH * GBLK], F16, tag="junk")
    acc = sb.tile([128, B], F32, tag="acc")
    # dir1 folds
    f1 = sb.tile([128, NG, 64], F16, tag="f1")
    f2 = sb.tile([128, NG, 32], F16, tag="f2")
    f3 = sb.tile([128, NG, 16], F16, tag="f3")
    f4 = sb.tile([128, NG, 8], F16, tag="f4")
    d1sb = sb.tile([128, NG, 1], F16, tag="d1sb")

    nc.sync.dma_start(qb_sb[:], qb_in)
    for g0 in range(0, NG, DMA_GRP):
        gs = slice(g0, g0 + DMA_GRP)
        eng = nc.sync if (g0 // DMA_GRP) % 2 == 0 else nc.scalar
        eng.dma_start(w_sb[:, gs], w_in[:, gs])
        eng.dma_start(m_sb[:, gs], m_in[:, gs])

    for g in range(NG):
        nc.tensor.matmul(
            ps[:, g], w_sb[:, g], m_sb[:, g], start=True, stop=True
        )
        nc.scalar.activation(
            gsb[:, g], ps[:, g], IDENT, bias=qb_sb[:, g : g + 1]
        )

    # one batched per-group transpose per half (overlaps the other half)
    for h in range(B):
        hs = slice(h * NGH, (h + 1) * NGH)
        eng = nc.sync if h == 0 else nc.scalar
        eng.dma_start_transpose(tt[:, hs], gsb[:, hs])

    # dir1: fold points (free axis of gsb) 128 -> 1, full width
    nc.vector.tensor_tensor(f1[:], gsb[:, :, 0:64], gsb[:, :, 64:128], op=MAX)
    nc.vector.tensor_tensor(f2[:], f1[:, :, 0:32], f1[:, :, 32:64], op=MAX)
    nc.vector.tensor_tensor(f3[:], f2[:, :, 0:16], f2[:, :, 16:32], op=MAX)
    nc.vector.tensor_tensor(f4[:], f3[:, :, 0:8], f3[:, :, 8:16], op=MAX)
    nc.vector.tensor_reduce(out=d1sb[:], in_=f4[:], op=MAX, axis=AX.X)

    # dir2: fold own-block slots (last axis of tt, viewed [.., GBLK, L]) 8 -> 1
    ttv = tt[:].rearrange("p g (c j) -> p g c j", j=L)
    nc.vector.tensor_tensor(l1[:], ttv[:, :, :, 0:4], ttv[:, :, :, 4:8], op=MAX)
    nc.vector.tensor_tensor(l2[:], l1[:, :, :, 0:2], l1[:, :, :, 2:4], op=MAX)
    nc.vector.tensor_tensor(l3[:], l2[:, :, :, 0:1], l2[:, :, :, 1:2], op=MAX)
    for h in range(B):
        hs = slice(h * NGH, (h + 1) * NGH)
        nc.scalar.activation(
            junk[:], l3[:, hs, :, 0], COPY, accum_out=acc[:, h : h + 1]
        )

    nc.sync.dma_start(d1_out, d1sb[:])
    nc.sync.dma_start(d2_out, acc[:])
    ctx.close()


_CACHE = {}


def _get_compiled():
    if "nc" in _CACHE:
        return _CACHE["nc"]
    nc = bacc.Bacc(
        "TRN2",
        target_bir_lowering=False,
        debug=False,
        enable_asserts=False,
        num_devices=NCORES,
    )
    w_in = nc.dram_tensor("w", [KK, NG, 128], F32, kind="ExternalInput").ap()
    m_in = nc.dram_tensor("m", [KK, NG, 128], F32, kind="ExternalInput").ap()
    qb_in = nc.dram_tensor("qb", [128, NG], F32, kind="ExternalInput").ap()
    d1_out = nc.dram_tensor("d1", [128, NG, 1], F16, kind="ExternalOutput").ap()
    d2_out = nc.dram_tensor("d2", [128, B], F32, kind="ExternalOutput").ap()

    with tile.TileContext(nc) as tc:
        _build_kernel(nc, tc, w_in, m_in, qb_in, d1_out, d2_out)
    nc.compile()
    _CACHE["nc"] = nc
    return nc


def _prep(target: np.ndarray, bin_edges: np.ndarray):
    """Host prep: sort, block metadata, packed W/M/bias arrays.

    Returns (in_maps, meta, cts_sorted); meta[c]["lo"/"ln"] are [NG, GBLK]
    (ln == 0 marks a padding block slot)."""
    target = np.asarray(target, dtype=np.float32).reshape(B, N)
    edges = np.asarray(bin_edges, dtype=np.float64)

    w_all = np.zeros((NCORES, KK, NG, 128), np.float32)
    m_all = np.zeros((NCORES, KK, NG, 128), np.float32)
    qb_all = np.zeros((NCORES, 128, NG), np.float32)
    meta = [
        {"lo": np.zeros((NG, GBLK), np.int64), "ln": np.zeros((NG, GBLK), np.int64)}
        for _ in range(NCORES)
    ]
    cts_sorted = []

    for b in range(B):
        pts = np.sort(target[b])
        cts = np.sort(0.5 * (edges[b, :-1] + edges[b, 1:]))
        cts_sorted.append(cts)
        pts64 = pts.astype(np.float64)

        t0s = pts64[0::BLK]
        t1s = pts64[BLK - 1 :: BLK]
        tprev = np.concatenate(([-np.inf], t1s[:-1]))
        tnext = np.concatenate((t0s[1:], [np.inf]))

        lo = np.minimum(
            np.searchsorted(cts, tprev, side="right"),
            np.searchsorted(cts, t0s, side="right") - 1,
        )
        lo = np.maximum(lo, 0)
        hi = np.maximum(
            np.searchsorted(cts, tnext, side="left") - 1,
            np.searchsorted(cts, t1s, side="left"),
        )
        hi = np.minimum(hi, K - 1)
        ln = hi - lo + 1
        assert ln.max() <= L, f"candidate run {ln.max()} exceeds L={L}"

        a = t0s
        x = (S * (pts64.reshape(NBLK_B, BLK) - a[:, None])).astype(np.float32)
        row0 = (-(x.astype(np.float64) ** 2)).astype(np.float32)   # [600, 128]

        idx = lo[:, None] + np.arange(L)[None, :]
        valid = np.arange(L)[None, :] < ln[:, None]
        idxc = np.clip(idx, 0, K - 1)
        y = (S * (cts[idxc] - a[:, None])).astype(np.float32)      # [600, L]
        wy = np.where(valid, 2.0 * y.astype(np.float64), 0.0).astype(np.float32)
        wq = np.where(valid, -(y.astype(np.float64) ** 2), -FAR).astype(np.float32)

        for c in range(NCORES):
            blks = np.arange(c * BPB, (c + 1) * BPB)
            for s_i, gblk in enumerate(blks):
                g = b * NGH + s_i // GBLK
                gb = s_i % GBLK
                m_all[c, 2 * gb + 0, g, :] = row0[gblk]
                m_all[c, 2 * gb + 1, g, :] = x[gblk]
                # slot j of block gb at output partition 8*gb + j; the HW
                # DMA transpose is a plain transpose (tt[p, f] = gsb[f, p]),
                # so this groups a block's slots contiguously along the last
                # axis of tt. (CoreSim models a different XBAR permutation —
                # HW behavior probed and confirmed in probe.py.)
                cols = slice(L * gb, L * gb + L)
                w_all[c, 2 * gb + 0, g, cols] = 1.0
                w_all[c, 2 * gb + 1, g, cols] = wy[gblk]
                qb_all[c, cols, g] = wq[gblk]
                meta[c]["lo"][g, gb] = lo[gblk]
                meta[c]["ln"][g, gb] = ln[gblk]

    in_maps = [
        {
            "w": np.ascontiguousarray(w_all[c]),
            "m": np.ascontiguousarray(m_all[c]),
            "qb": np.ascontiguousarray(qb_all[c]),
        }
        for c in range(NCORES)
    ]
    return in_maps, meta, cts_sorted


def _combine(results, meta):
    d2_tot = np.zeros(B, np.float64)
    gmax = np.full((B, K), -np.inf)
    for c, res in enumerate(results):
        d2 = np.asarray(res["d2"], np.float64)               # [128, B]
        d2_tot += d2.sum(axis=0)
        d1 = np.asarray(res["d1"], np.float64)               # [128, NG, 1]
        lo, ln = meta[c]["lo"], meta[c]["ln"]
        for g in range(NG):
            h, gh = divmod(g, NGH)
            for gb in range(GBLK):
                ll = ln[g, gb]
                if ll == 0:
                    continue
                li = lo[g, gb]
                vals = d1[L * gb : L * gb + ll, g, 0]
                np.maximum.at(gmax[h], np.arange(li, li + ll), vals)
    assert np.isfinite(gmax).all(), "uncovered center in dir1 combine"
    dir2 = -d2_tot / (S * S)
    dir1 = (-gmax / (S * S)).sum(axis=1)
    return np.float32((dir1 + dir2).mean())


def kernel(target: np.ndarray, bin_edges: np.ndarray) -> np.ndarray:
    in_maps, meta, _ = _prep(target, bin_edges)
    nc = _get_compiled()
    res = run_bass_kernel_spmd(nc, in_maps, list(range(NCORES))).results
    out = _combine(res, meta)
    return np.asarray(out, dtype=np.float32)


# revision 17
# speedup vs baseline: 2.5955x; 1.0220x over previous
"""Chamfer distance v11: block-sparse KNN + block-diagonal grouped matmuls.

Host sorts points per batch; 128-point sorted blocks each have a contiguous
run of <=8 candidate sorted centers (data max 6). 16 blocks are packed into
one fp32 matmul via a block-diagonal stationary; slot j of block gb lands at
output partition 16*j+gb (matches the DMA-transpose layout). Grid value:
    G = -(x^2)*1 + x*(2y) + bias(-y^2),  x = S(t - a_blk), y = S(c - a_blk)
K = 2 rows per block * 16 blocks = 32; the -y^2 term is applied as the
per-partition bias of the ScalarE squash (Identity activation), so it needs
no matmul rows. One LdWeights per 16 blocks instead of one per block (the
v10 killer: 300 LdWeights = 64us).

Reductions per group g (PSUM [128 blockslots, 128 points]):
  squash: ScalarE Identity+bias -> f16 SBUF
  dir1 (per-center min over points): free-axis max-folds (Pool+DVE) -> [128, NG]
  dir2 (per-point min over centers): DMA-transpose [128,128], DVE max-tree over
        own block's 8 slots -> per-point maxes, ScalarE accum-sum per batch.
Host: sorting, block metadata, final tiny combines (exact, f64).
"""

import sys

if "/opt/trn_rl_repo" not in sys.path:
    sys.path.insert(0, "/opt/trn_rl_repo")

import numpy as np

import concourse.bass as bass
import concourse.tile as tile
from concourse import bacc, mybir
from concourse.bass_utils import run_bass_kernel_spmd

B = 2
N = 76800
E = 257
K = 256
NCORES = 8
BLK = 128
NBLK_B = N // BLK          # 600 blocks per batch
BPB = NBLK_B // NCORES     # 75 blocks per (core, batch)
GBLK = 16                  # blocks per matmul group
NGH = 5                    # groups per batch-half (5*16 = 80 slots, 75 real)
NG = B * NGH               # 10 groups per core
L = 8                      # candidate-center slots per block (data max is 6)
KK = 2 * GBLK              # contraction rows per group matmul (s row + x row)
S = 1024.0
FAR = 3.0e4

F32 = mybir.dt.float32
F16 = mybir.dt.float16
MAX = mybir.AluOpType.max
COPY = mybir.ActivationFunctionType.Copy
IDENT = mybir.ActivationFunctionType.Identity
AX = mybir.AxisListType

DMA_GRP = 2                # groups per input DMA chunk


def _build_kernel(nc, tc, wm_in, qb_in, d1_out, d2_out):
    from contextlib import ExitStack

    ctx = ExitStack()
    sb = ctx.enter_context(tc.tile_pool(name="sb", bufs=1))
    psum_pool = ctx.enter_context(tc.tile_pool(name="ps", bufs=1, space="PSUM"))

    wm_sb = sb.tile([KK, 2, NG, 128], F32, tag="wm")
    qb_sb = sb.tile([128, NG], F32, tag="qb")
    ps = psum_pool.tile([128, 4, 512], F32, tag="grid")   # 4 banks, ping-pong
    gsb = sb.tile([128, NG, 128], F16, tag="gsb")
    tt = sb.tile([128, NG, 128], F16, tag="tt")
    # dir2 tree (per half h: groups h*NGH..h*NGH+NGH)
    l1 = sb.tile([128, NG, GBLK, 4], F16, tag="l1")
    l2 = sb.tile([128, NG, GBLK, 2], F16, tag="l2")
    l3 = sb.tile([128, NG, GBLK, 1], F16, tag="l3")
    junk = sb.tile([128, NGH * GBLK], F16, tag="junk")
    acc = sb.tile([128, B], F32, tag="acc")
    # dir1 folds
    f1 = sb.tile([128, NG, 64], F16, tag="f1")
    f2 = sb.tile([128, NG, 32], F16, tag="f2")
    f3 = sb.tile([128, NG, 16], F16, tag="f3")
    f4 = sb.tile([128, NG, 8], F16, tag="f4")
    d1sb = sb.tile([128, NG, 1], F16, tag="d1sb")

    nc.scalar.dma_start(qb_sb[:], qb_in)
    nc.sync.dma_start(wm_sb[:], wm_in)

    def emit_half(h):
        hs = slice(h * NGH, (h + 1) * NGH)
        for g in range(h * NGH, (h + 1) * NGH):
            bank = g % 4
            nc.tensor.matmul(
                ps[:, bank, 0:128], wm_sb[:, 0, g], wm_sb[:, 1, g],
                start=True, stop=True,
            )
            nc.scalar.activation(
                gsb[:, g], ps[:, bank, 0:128], IDENT, bias=qb_sb[:, g : g + 1]
            )
        eng = nc.sync if h == 0 else nc.scalar
        eng.dma_start_transpose(tt[:, hs], gsb[:, hs])
        # dir1: fold points (free axis of gsb) 128 -> 1
        nc.vector.tensor_tensor(
            f1[:, hs], gsb[:, hs, 0:64], gsb[:, hs, 64:128], op=MAX
        )
        nc.vector.tensor_tensor(
            f2[:, hs], f1[:, hs, 0:32], f1[:, hs, 32:64], op=MAX
        )
        nc.vector.tensor_tensor(
            f3[:, hs], f2[:, hs, 0:16], f2[:, hs, 16:32], op=MAX
        )
        nc.vector.tensor_tensor(
            f4[:, hs], f3[:, hs, 0:8], f3[:, hs, 8:16], op=MAX
        )
        nc.vector.tensor_reduce(
            out=d1sb[:, hs], in_=f4[:, hs], op=MAX, axis=AX.X
        )
        # dir2: fold own-block slots (last axis of tt viewed [.., GBLK, L])
        ttv = tt[:].rearrange("p g (c j) -> p g c j", j=L)
        nc.vector.tensor_tensor(
            l1[:, hs], ttv[:, hs, :, 0:4], ttv[:, hs, :, 4:8], op=MAX
        )
        nc.vector.tensor_tensor(
            l2[:, hs], l1[:, hs, :, 0:2], l1[:, hs, :, 2:4], op=MAX
        )
        nc.vector.tensor_tensor(
            l3[:, hs], l2[:, hs, :, 0:1], l2[:, hs, :, 1:2], op=MAX
        )
        nc.scalar.activation(
            junk[:], l3[:, hs, :, 0], COPY, accum_out=acc[:, h : h + 1]
        )

    for h in range(B):
        emit_half(h)

    nc.sync.dma_start(d1_out, d1sb[:])
    nc.sync.dma_start(d2_out, acc[:])
    ctx.close()


_CACHE = {}


def _get_compiled():
    if "nc" in _CACHE:
        return _CACHE["nc"]
    nc = bacc.Bacc(
        "TRN2",
        target_bir_lowering=False,
        debug=False,
        enable_asserts=False,
        num_devices=NCORES,
    )
    wm_in = nc.dram_tensor("wm", [KK, 2, NG, 128], F32, kind="ExternalInput").ap()
    qb_in = nc.dram_tensor("qb", [128, NG], F32, kind="ExternalInput").ap()
    d1_out = nc.dram_tensor("d1", [128, NG, 1], F16, kind="ExternalOutput").ap()
    d2_out = nc.dram_tensor("d2", [128, B], F32, kind="ExternalOutput").ap()

    with tile.TileContext(nc) as tc:
        _build_kernel(nc, tc, wm_in, qb_in, d1_out, d2_out)
    nc.compile()
    _CACHE["nc"] = nc
    return nc


def _prep(target: np.ndarray, bin_edges: np.ndarray):
    """Host prep: sort, block metadata, packed W/M/bias arrays.

    Returns (in_maps, meta, cts_sorted); meta[c]["lo"/"ln"] are [NG, GBLK]
    (ln == 0 marks a padding block slot)."""
    target = np.asarray(target, dtype=np.float32).reshape(B, N)
    edges = np.asarray(bin_edges, dtype=np.float64)

    wm_all = np.zeros((NCORES, KK, 2, NG, 128), np.float32)
    w_all = wm_all[:, :, 0]
    m_all = wm_all[:, :, 1]
    qb_all = np.zeros((NCORES, 128, NG), np.float32)
    meta = [
        {"lo": np.zeros((NG, GBLK), np.int64), "ln": np.zeros((NG, GBLK), np.int64)}
        for _ in range(NCORES)
    ]
    cts_sorted = []

    for b in range(B):
        pts = np.sort(target[b])
        cts = np.sort(0.5 * (edges[b, :-1] + edges[b, 1:]))
        cts_sorted.append(cts)
        pts64 = pts.astype(np.float64)

        t0s = pts64[0::BLK]
        t1s = pts64[BLK - 1 :: BLK]
        tprev = np.concatenate(([-np.inf], t1s[:-1]))
        tnext = np.concatenate((t0s[1:], [np.inf]))

        lo = np.minimum(
            np.searchsorted(cts, tprev, side="right"),
            np.searchsorted(cts, t0s, side="right") - 1,
        )
        lo = np.maximum(lo, 0)
        hi = np.maximum(
            np.searchsorted(cts, tnext, side="left") - 1,
            np.searchsorted(cts, t1s, side="left"),
        )
        hi = np.minimum(hi, K - 1)
        ln = hi - lo + 1
        assert ln.max() <= L, f"candidate run {ln.max()} exceeds L={L}"

        a = t0s
        x = (S * (pts64.reshape(NBLK_B, BLK) - a[:, None])).astype(np.float32)
        row0 = (-(x.astype(np.float64) ** 2)).astype(np.float32)   # [600, 128]

        idx = lo[:, None] + np.arange(L)[None, :]
        valid = np.arange(L)[None, :] < ln[:, None]
        idxc = np.clip(idx, 0, K - 1)
        y = (S * (cts[idxc] - a[:, None])).astype(np.float32)      # [600, L]
        wy = np.where(valid, 2.0 * y.astype(np.float64), 0.0).astype(np.float32)
        wq = np.where(valid, -(y.astype(np.float64) ** 2), -FAR).astype(np.float32)

        for c in range(NCORES):
            blks = np.arange(c * BPB, (c + 1) * BPB)
            for s_i, gblk in enumerate(blks):
                g = b * NGH + s_i // GBLK
                gb = s_i % GBLK
                m_all[c, 2 * gb + 0, g, :] = row0[gblk]
                m_all[c, 2 * gb + 1, g, :] = x[gblk]
                # slot j of block gb at output partition 8*gb + j; the HW
                # DMA transpose is a plain transpose (tt[p, f] = gsb[f, p]),
                # so this groups a block's slots contiguously along the last
                # axis of tt. (CoreSim models a different XBAR permutation —
                # HW behavior probed and confirmed in probe.py.)
                cols = slice(L * gb, L * gb + L)
                w_all[c, 2 * gb + 0, g, cols] = 1.0
                w_all[c, 2 * gb + 1, g, cols] = wy[gblk]
                qb_all[c, cols, g] = wq[gblk]
                meta[c]["lo"][g, gb] = lo[gblk]
                meta[c]["ln"][g, gb] = ln[gblk]

    in_maps = [
        {
            "wm": np.ascontiguousarray(wm_all[c]),
            "qb": np.ascontiguousarray(qb_all[c]),
        }
        for c in range(NCORES)
    ]
    return in_maps, meta, cts_sorted


def _combine(results, meta):
    d2_tot = np.zeros(B, np.float64)
    gmax = np.full((B, K), -np.inf)
    for c, res in enumerate(results):
        d2 = np.asarray(res["d2"], np.float64)               # [128, B]
        d2_tot += d2.sum(axis=0)
        d1 = np.asarray(res["d1"], np.float64)               # [128, NG, 1]
        lo, ln = meta[c]["lo"], meta[c]["ln"]
        for g in range(NG):
            h, gh = divmod(g, NGH)
            for gb in range(GBLK):
                ll = ln[g, gb]
                if ll == 0:
                    continue
                li = lo[g, gb]
                vals = d1[L * gb : L * gb + ll, g, 0]
                np.maximum.at(gmax[h], np.arange(li, li + ll), vals)
    assert np.isfinite(gmax).all(), "uncovered center in dir1 combine"
    dir2 = -d2_tot / (S * S)
    dir1 = (-gmax / (S * S)).sum(axis=1)
    return np.float32((dir1 + dir2).mean())


def kernel(target: np.ndarray, bin_edges: np.ndarray) -> np.ndarray:
    in_maps, meta, _ = _prep(target, bin_edges)
    nc = _get_compiled()
    res = run_bass_kernel_spmd(nc, in_maps, list(range(NCORES))).results
    out = _combine(res, meta)
    return np.asarray(out, dtype=np.float32)


# revision 19
# speedup vs baseline: 3.2019x; 1.2336x over previous
"""Chamfer distance v14: block-sparse KNN, bf16-limb grouped matmuls, no ScalarE.

Host sorts points per batch; 128-point sorted blocks each have a contiguous
run of <=8 candidate sorted centers (data max 6). 16 blocks pack into one
bf16 matmul via a block-diagonal stationary; slot j of block gb sits at
output partition 8*gb+j. Grid value (negated scaled squared distance):
    G = sh + sl + xh*2yh + xh*2yl + xl*2yh + bias(-y^2)
with x = S(t - a_blk) split into bf16 limbs (xh, xl), s = -x^2 limbs
(sh, sl), y = S(c - a_blk) limbs (yh, yl); products are exact in bf16 x
bf16 -> f32. K = 5 rows per block * 16 blocks = 80. The -y^2 term rides the
per-partition scalar of the DVE squash, so ScalarE is never used (saves the
1.3us ACT_TABLE_LOAD and frees its DMA queue).

Per group g (PSUM [128 blockslots, 128 points], 4 PSUM banks ping-ponged):
  squash: DVE tensor_scalar add(-y^2) -> f16 SBUF
  dir1 (per-center min over points): DVE free-axis max-folds -> [128, NG]
  dir2 (per-point min over centers): batched DMA-transpose, DVE max-tree over
        own block's 8 slots, DVE reduce-add per batch.
Host: sorting, block metadata (searchsorted of block bounds), final combines.
"""

import sys

if "/opt/trn_rl_repo" not in sys.path:
    sys.path.insert(0, "/opt/trn_rl_repo")

import numpy as np
import ml_dtypes

import concourse.bass as bass
import concourse.tile as tile
from concourse import bacc, mybir
from concourse.bass_utils import run_bass_kernel_spmd

B = 2
N = 76800
E = 257
K = 256
NCORES = 8
BLK = 128
NBLK_B = N // BLK          # 600 blocks per batch
BPB = NBLK_B // NCORES     # 75 blocks per (core, batch)
GBLK = 16                  # blocks per matmul group
NGH = 5                    # groups per batch-half (5*16 = 80 slots, 75 real)
NG = B * NGH               # 10 groups per core
L = 8                      # candidate-center slots per block (data max is 6)
KR = 5                     # limb rows per block
KK = KR * GBLK             # 80 contraction rows per group matmul
S = 1024.0
FAR = 3.0e4

F32 = mybir.dt.float32
F16 = mybir.dt.float16
BF16 = mybir.dt.bfloat16
MAX = mybir.AluOpType.max
ADD = mybir.AluOpType.add
AX = mybir.AxisListType
BF = ml_dtypes.bfloat16


def _build_kernel(nc, tc, wm_in, qb_in, d1_out, d2_out):
    from contextlib import ExitStack

    ctx = ExitStack()
    sb = ctx.enter_context(tc.tile_pool(name="sb", bufs=1))
    psum_pool = ctx.enter_context(tc.tile_pool(name="ps", bufs=1, space="PSUM"))

    wm_sb = sb.tile([KK, 2, NG, 128], BF16, tag="wm")
    qb_sb = sb.tile([128, NG], F32, tag="qb")
    # one full PSUM bank per tile so dependency tracking stays per-group
    psb = [
        psum_pool.tile([128, 512], F32, tag=f"ps{i}", name=f"ps{i}")
        for i in range(4)
    ]
    gsb = sb.tile([128, NG, 128], F16, tag="gsb")
    tt = sb.tile([128, NG, 128], F16, tag="tt")
    l1 = sb.tile([128, NG, GBLK, 4], F16, tag="l1")
    l2 = sb.tile([128, NG, GBLK, 2], F16, tag="l2")
    l3 = sb.tile([128, NG, GBLK, 1], F16, tag="l3")
    acc = sb.tile([128, B], F32, tag="acc")
    f1 = sb.tile([128, NG, 64], F16, tag="f1")
    f2 = sb.tile([128, NG, 32], F16, tag="f2")
    f3 = sb.tile([128, NG, 16], F16, tag="f3")
    f4 = sb.tile([128, NG, 8], F16, tag="f4")
    d1sb = sb.tile([128, NG, 1], F16, tag="d1sb")

    ttv = tt[:].rearrange("p g (c j) -> p g c j", j=L)

    # stream inputs in 2-group chunks, alternating HWDGE queues
    nc.scalar.dma_start(qb_sb[:], qb_in)
    for i, g0 in enumerate(range(0, NG, 2)):
        gs = slice(g0, g0 + 2)
        eng = nc.sync if i % 2 == 0 else nc.scalar
        eng.dma_start(wm_sb[:, :, gs], wm_in[:, :, gs])

    def mm(g):
        ps = psb[g % 4]
        nc.tensor.matmul(
            ps[:, 0:128], wm_sb[:, 0, g], wm_sb[:, 1, g], start=True, stop=True
        )
        return ps

    def squash(g, ps):
        nc.vector.tensor_scalar(
            gsb[:, g], ps[:, 0:128], qb_sb[:, g : g + 1], None, op0=ADD
        )

    def fchain(h):
        hs = slice(h * NGH, (h + 1) * NGH)
        nc.vector.tensor_tensor(
            f1[:, hs], gsb[:, hs, 0:64], gsb[:, hs, 64:128], op=MAX
        )
        nc.vector.tensor_tensor(f2[:, hs], f1[:, hs, 0:32], f1[:, hs, 32:64], op=MAX)
        nc.vector.tensor_tensor(f3[:, hs], f2[:, hs, 0:16], f2[:, hs, 16:32], op=MAX)
        nc.vector.tensor_tensor(f4[:, hs], f3[:, hs, 0:8], f3[:, hs, 8:16], op=MAX)
        nc.vector.tensor_reduce(out=d1sb[:, hs], in_=f4[:, hs], op=MAX, axis=AX.X)

    def ltree(h):
        hs = slice(h * NGH, (h + 1) * NGH)
        nc.vector.tensor_tensor(
            l1[:, hs], ttv[:, hs, :, 0:4], ttv[:, hs, :, 4:8], op=MAX
        )
        nc.vector.tensor_tensor(l2[:, hs], l1[:, hs, :, 0:2], l1[:, hs, :, 2:4], op=MAX)
        nc.vector.tensor_tensor(l3[:, hs], l2[:, hs, :, 0:1], l2[:, hs, :, 1:2], op=MAX)
        nc.vector.tensor_reduce(
            out=acc[:, h : h + 1], in_=l3[:, hs, :, 0], op=ADD, axis=AX.XY
        )

    # pipeline: PE runs ahead on the 4 psum banks; DVE squashes trail;
    # per-half tails (transpose -> trees) overlap the other half's matmuls
    pss = {}
    for g in range(5):
        pss[g] = mm(g)
    for g in range(5):
        squash(g, pss[g])
        pss[5 + g] = mm(5 + g)
    nc.sync.dma_start_transpose(tt[:, 0:NGH], gsb[:, 0:NGH])
    fchain(0)
    for g in range(5, 9):
        squash(g, pss[g])
    nc.scalar.dma_start_transpose(tt[:, NGH : NGH + 4], gsb[:, NGH : NGH + 4])
    ltree(0)
    squash(9, pss[9])
    nc.sync.dma_start_transpose(tt[:, 9:10], gsb[:, 9:10])
    fchain(1)
    ltree(1)

    nc.scalar.dma_start(d1_out, d1sb[:])
    nc.scalar.dma_start(d2_out, acc[:])
    ctx.close()


_CACHE = {}


def _get_compiled():
    if "nc" in _CACHE:
        return _CACHE["nc"]
    nc = bacc.Bacc(
        "TRN2",
        target_bir_lowering=False,
        debug=False,
        enable_asserts=False,
        num_devices=NCORES,
    )
    wm_in = nc.dram_tensor("wm", [KK, 2, NG, 128], BF16, kind="ExternalInput").ap()
    qb_in = nc.dram_tensor("qb", [128, NG], F32, kind="ExternalInput").ap()
    d1_out = nc.dram_tensor("d1", [128, NG, 1], F16, kind="ExternalOutput").ap()
    d2_out = nc.dram_tensor("d2", [128, B], F32, kind="ExternalOutput").ap()

    with tile.TileContext(nc) as tc:
        _build_kernel(nc, tc, wm_in, qb_in, d1_out, d2_out)
    nc.compile()
    _CACHE["nc"] = nc
    return nc


def _limbs(v):
    hi = v.astype(BF).astype(np.float64)
    lo = (v - hi).astype(BF).astype(np.float64)
    return hi, lo


def _prep(target: np.ndarray, bin_edges: np.ndarray):
    """Host prep: sort, block metadata, packed bf16 W/M rows + f32 bias."""
    target = np.asarray(target, dtype=np.float32).reshape(B, N)
    edges = np.asarray(bin_edges, dtype=np.float64)

    wm_all = np.zeros((NCORES, KK, 2, NG, 128), BF)
    qb_all = np.zeros((NCORES, 128, NG), np.float32)
    meta = [
        {"lo": np.zeros((NG, GBLK), np.int64), "ln": np.zeros((NG, GBLK), np.int64)}
        for _ in range(NCORES)
    ]
    cts_sorted = []

    for b in range(B):
        pts = np.sort(target[b])
        cts = np.sort(0.5 * (edges[b, :-1] + edges[b, 1:]))
        cts_sorted.append(cts)
        pts64 = pts.astype(np.float64)

        t0s = pts64[0::BLK]
        t1s = pts64[BLK - 1 :: BLK]
        tprev = np.concatenate(([-np.inf], t1s[:-1]))
        tnext = np.concatenate((t0s[1:], [np.inf]))

        lo = np.minimum(
            np.searchsorted(cts, tprev, side="right"),
            np.searchsorted(cts, t0s, side="right") - 1,
        )
        lo = np.maximum(lo, 0)
        hi = np.maximum(
            np.searchsorted(cts, tnext, side="left") - 1,
            np.searchsorted(cts, t1s, side="left"),
        )
        hi = np.minimum(hi, K - 1)
        ln = hi - lo + 1
        assert ln.max() <= L, f"candidate run {ln.max()} exceeds L={L}"

        a = t0s
        x = S * (pts64.reshape(NBLK_B, BLK) - a[:, None])       # [600, 128]
        xh, xl = _limbs(x)
        sh, sl = _limbs(-(x * x))

        idx = lo[:, None] + np.arange(L)[None, :]
        valid = np.arange(L)[None, :] < ln[:, None]
        idxc = np.clip(idx, 0, K - 1)
        y = S * (cts[idxc] - a[:, None])                        # [600, L]
        yh, yl = _limbs(y)
        w2yh = np.where(valid, 2.0 * yh, 0.0)
        w2yl = np.where(valid, 2.0 * yl, 0.0)
        qbv = np.where(valid, -((yh + yl) ** 2), -FAR).astype(np.float32)

        for c in range(NCORES):
            blks = np.arange(c * BPB, (c + 1) * BPB)
            for s_i, gblk in enumerate(blks):
                g = b * NGH + s_i // GBLK
                gb = s_i % GBLK
                r = KR * gb
                # moving rows (t-side)
                wm_all[c, r + 0, 1, g, :] = sh[gblk]
                wm_all[c, r + 1, 1, g, :] = sl[gblk]
                wm_all[c, r + 2, 1, g, :] = xh[gblk]
                wm_all[c, r + 3, 1, g, :] = xh[gblk]
                wm_all[c, r + 4, 1, g, :] = xl[gblk]
                # stationary cols (c-side), block diagonal at cols 8*gb+j
                cols = slice(L * gb, L * gb + L)
                wm_all[c, r + 0, 0, g, cols] = 1.0
                wm_all[c, r + 1, 0, g, cols] = 1.0
                wm_all[c, r + 2, 0, g, cols] = w2yh[gblk]
                wm_all[c, r + 3, 0, g, cols] = w2yl[gblk]
                wm_all[c, r + 4, 0, g, cols] = w2yh[gblk]
                qb_all[c, cols, g] = qbv[gblk]
                meta[c]["lo"][g, gb] = lo[gblk]
                meta[c]["ln"][g, gb] = ln[gblk]

    in_maps = [
        {
            "wm": np.ascontiguousarray(wm_all[c]),
            "qb": np.ascontiguousarray(qb_all[c]),
        }
        for c in range(NCORES)
    ]
    return in_maps, meta, cts_sorted


def _combine(results, meta):
    d2_tot = np.zeros(B, np.float64)
    gmax = np.full((B, K), -np.inf)
    for c, res in enumerate(results):
        d2 = np.asarray(res["d2"], np.float64)               # [128, B]
        d2_tot += d2.sum(axis=0)
        d1 = np.asarray(res["d1"], np.float64)               # [128, NG, 1]
        lo, ln = meta[c]["lo"], meta[c]["ln"]
        for g in range(NG):
            h = g // NGH
            for gb in range(GBLK):
                ll = ln[g, gb]
                if ll == 0:
                    continue
                li = lo[g, gb]
                vals = d1[L * gb : L * gb + ll, g, 0]
                np.maximum.at(gmax[h], np.arange(li, li + ll), vals)
    assert np.isfinite(gmax).all(), "uncovered center in dir1 combine"
    dir2 = -d2_tot / (S * S)
    dir1 = (-gmax / (S * S)).sum(axis=1)
    return np.float32((dir1 + dir2).mean())


def kernel(target: np.ndarray, bin_edges: np.ndarray) -> np.ndarray:
    in_maps, meta, _ = _prep(target, bin_edges)
    nc = _get_compiled()
    res = run_bass_kernel_spmd(nc, in_maps, list(range(NCORES))).results
    out = _combine(res, meta)
    return np.asarray(out, dtype=np.float32)


# revision 20
# speedup vs baseline: 3.6568x; 1.1421x over previous
"""Chamfer distance v14: block-sparse KNN, bf16-limb grouped matmuls, no ScalarE.

Host sorts points per batch; 128-point sorted blocks each have a contiguous
run of <=8 candidate sorted centers (data max 6). 16 blocks pack into one
bf16 matmul via a block-diagonal stationary; slot j of block gb sits at
output partition 8*gb+j. Grid value (negated scaled squared distance):
    G = sh + sl + xh*2yh + xh*2yl + xl*2yh + bias(-y^2)
with x = S(t - a_blk) split into bf16 limbs (xh, xl), s = -x^2 limbs
(sh, sl), y = S(c - a_blk) limbs (yh, yl); products are exact in bf16 x
bf16 -> f32. K = 5 rows per block * 16 blocks = 80. The -y^2 term rides the
per-partition scalar of the DVE squash, so ScalarE is never used (saves the
1.3us ACT_TABLE_LOAD and frees its DMA queue).

Per group g (PSUM [128 blockslots, 128 points], 4 PSUM banks ping-ponged):
  squash: DVE tensor_scalar add(-y^2) -> f16 SBUF
  dir1 (per-center min over points): DVE free-axis max-folds -> [128, NG]
  dir2 (per-point min over centers): batched DMA-transpose, DVE max-tree over
        own block's 8 slots, DVE reduce-add per batch.
Host: sorting, block metadata (searchsorted of block bounds), final combines.
"""

import sys

if "/opt/trn_rl_repo" not in sys.path:
    sys.path.insert(0, "/opt/trn_rl_repo")

import numpy as np
import ml_dtypes

import concourse.bass as bass
import concourse.tile as tile
from concourse import bacc, mybir
from concourse.bass_utils import run_bass_kernel_spmd

B = 2
N = 76800
E = 257
K = 256
NCORES = 8
BLK = 128
NBLK_B = N // BLK          # 600 blocks per batch
BPB = NBLK_B // NCORES     # 75 blocks per (core, batch)
GBLK = 16                  # blocks per matmul group
NGH = 5                    # groups per batch-half (5*16 = 80 slots, 75 real)
NG = B * NGH               # 10 groups per core
L = 8                      # candidate-center slots per block (data max is 6)
KR = 5                     # limb rows per block
KK = KR * GBLK             # 80 contraction rows per group matmul
S = 1024.0
FAR = 3.0e4

F32 = mybir.dt.float32
F16 = mybir.dt.float16
BF16 = mybir.dt.bfloat16
MAX = mybir.AluOpType.max
ADD = mybir.AluOpType.add
AX = mybir.AxisListType
BF = ml_dtypes.bfloat16


def _build_kernel(nc, tc, wm_in, qb_in, d1_out, d2_out):
    from contextlib import ExitStack

    ctx = ExitStack()
    sb = ctx.enter_context(tc.tile_pool(name="sb", bufs=1))
    psum_pool = ctx.enter_context(tc.tile_pool(name="ps", bufs=1, space="PSUM"))

    wm_sb = sb.tile([KK, 2, NG, 128], BF16, tag="wm")
    qb_sb = sb.tile([128, NG], F32, tag="qb")
    # one full PSUM bank per tile so dependency tracking stays per-group
    psb = [
        psum_pool.tile([128, 512], F32, tag=f"ps{i}", name=f"ps{i}")
        for i in range(4)
    ]
    gsb = sb.tile([128, NG, 128], F16, tag="gsb")
    tt = sb.tile([128, NG, 128], F16, tag="tt")
    l1 = sb.tile([128, NG, GBLK, 4], F16, tag="l1")
    l2 = sb.tile([128, NG, GBLK, 2], F16, tag="l2")
    l3 = sb.tile([128, NG, GBLK, 1], F16, tag="l3")
    acc = sb.tile([128, B], F32, tag="acc")
    f1 = sb.tile([128, NG, 64], F16, tag="f1")
    f2 = sb.tile([128, NG, 32], F16, tag="f2")
    f3 = sb.tile([128, NG, 16], F16, tag="f3")
    f4 = sb.tile([128, NG, 8], F16, tag="f4")
    d1sb = sb.tile([128, NG, 1], F16, tag="d1sb")

    ttv = tt[:].rearrange("p g (c j) -> p g c j", j=L)

    # stream inputs in 2-group chunks, alternating HWDGE queues
    nc.scalar.dma_start(qb_sb[:], qb_in)
    for i, g0 in enumerate(range(0, NG, 2)):
        gs = slice(g0, g0 + 2)
        eng = nc.sync if i % 2 == 0 else nc.scalar
        eng.dma_start(wm_sb[:, :, gs], wm_in[:, :, gs])

    def mm(g):
        ps = psb[g % 4]
        nc.tensor.matmul(
            ps[:, 0:128], wm_sb[:, 0, g], wm_sb[:, 1, g], start=True, stop=True
        )
        return ps

    def squash(g, ps):
        nc.vector.tensor_scalar(
            gsb[:, g], ps[:, 0:128], qb_sb[:, g : g + 1], None, op0=ADD
        )

    def fchain(h):
        hs = slice(h * NGH, (h + 1) * NGH)
        nc.vector.tensor_tensor(
            f1[:, hs], gsb[:, hs, 0:64], gsb[:, hs, 64:128], op=MAX
        )
        nc.vector.tensor_tensor(f2[:, hs], f1[:, hs, 0:32], f1[:, hs, 32:64], op=MAX)
        nc.vector.tensor_tensor(f3[:, hs], f2[:, hs, 0:16], f2[:, hs, 16:32], op=MAX)
        nc.vector.tensor_tensor(f4[:, hs], f3[:, hs, 0:8], f3[:, hs, 8:16], op=MAX)
        nc.vector.tensor_reduce(out=d1sb[:, hs], in_=f4[:, hs], op=MAX, axis=AX.X)

    def ltree(h):
        hs = slice(h * NGH, (h + 1) * NGH)
        nc.vector.tensor_tensor(
            l1[:, hs], ttv[:, hs, :, 0:4], ttv[:, hs, :, 4:8], op=MAX
        )
        nc.vector.tensor_tensor(l2[:, hs], l1[:, hs, :, 0:2], l1[:, hs, :, 2:4], op=MAX)
        nc.vector.tensor_tensor(l3[:, hs], l2[:, hs, :, 0:1], l2[:, hs, :, 1:2], op=MAX)
        nc.vector.tensor_reduce(
            out=acc[:, h : h + 1], in_=l3[:, hs, :, 0], op=ADD, axis=AX.XY
        )

    # pipeline: PE runs ahead on the 4 psum banks; DVE squashes trail.
    # Program-order constraint: squash(g) must be emitted BEFORE mm(g+4)
    # reuses its bank. Per-half tails overlap the other half's matmuls.
    pss = {}
    for g in range(4):
        pss[g] = mm(g)
    for g in range(6):
        squash(g, pss[g])
        pss[g + 4] = mm(g + 4)
    nc.sync.dma_start_transpose(tt[:, 0:NGH], gsb[:, 0:NGH])
    fchain(0)
    for g in range(6, 9):
        squash(g, pss[g])
    nc.scalar.dma_start_transpose(tt[:, NGH : NGH + 4], gsb[:, NGH : NGH + 4])
    ltree(0)
    squash(9, pss[9])
    nc.sync.dma_start_transpose(tt[:, 9:10], gsb[:, 9:10])
    fchain(1)
    ltree(1)

    nc.scalar.dma_start(d1_out, d1sb[:])
    nc.scalar.dma_start(d2_out, acc[:])
    ctx.close()


_CACHE = {}


def _get_compiled():
    if "nc" in _CACHE:
        return _CACHE["nc"]
    nc = bacc.Bacc(
        "TRN2",
        target_bir_lowering=False,
        debug=False,
        enable_asserts=False,
        num_devices=NCORES,
    )
    wm_in = nc.dram_tensor("wm", [KK, 2, NG, 128], BF16, kind="ExternalInput").ap()
    qb_in = nc.dram_tensor("qb", [128, NG], F32, kind="ExternalInput").ap()
    d1_out = nc.dram_tensor("d1", [128, NG, 1], F16, kind="ExternalOutput").ap()
    d2_out = nc.dram_tensor("d2", [128, B], F32, kind="ExternalOutput").ap()

    with tile.TileContext(nc) as tc:
        _build_kernel(nc, tc, wm_in, qb_in, d1_out, d2_out)
    nc.compile()
    _CACHE["nc"] = nc
    return nc


def _limbs(v):
    hi = v.astype(BF).astype(np.float64)
    lo = (v - hi).astype(BF).astype(np.float64)
    return hi, lo


def _prep(target: np.ndarray, bin_edges: np.ndarray):
    """Host prep: sort, block metadata, packed bf16 W/M rows + f32 bias."""
    target = np.asarray(target, dtype=np.float32).reshape(B, N)
    edges = np.asarray(bin_edges, dtype=np.float64)

    wm_all = np.zeros((NCORES, KK, 2, NG, 128), BF)
    qb_all = np.zeros((NCORES, 128, NG), np.float32)
    meta = [
        {"lo": np.zeros((NG, GBLK), np.int64), "ln": np.zeros((NG, GBLK), np.int64)}
        for _ in range(NCORES)
    ]
    cts_sorted = []

    for b in range(B):
        pts = np.sort(target[b])
        cts = np.sort(0.5 * (edges[b, :-1] + edges[b, 1:]))
        cts_sorted.append(cts)
        pts64 = pts.astype(np.float64)

        t0s = pts64[0::BLK]
        t1s = pts64[BLK - 1 :: BLK]
        tprev = np.concatenate(([-np.inf], t1s[:-1]))
        tnext = np.concatenate((t0s[1:], [np.inf]))

        lo = np.minimum(
            np.searchsorted(cts, tprev, side="right"),
            np.searchsorted(cts, t0s, side="right") - 1,
        )
        lo = np.maximum(lo, 0)
        hi = np.maximum(
            np.searchsorted(cts, tnext, side="left") - 1,
            np.searchsorted(cts, t1s, side="left"),
        )
        hi = np.minimum(hi, K - 1)
        ln = hi - lo + 1
        assert ln.max() <= L, f"candidate run {ln.max()} exceeds L={L}"

        a = t0s
        x = S * (pts64.reshape(NBLK_B, BLK) - a[:, None])       # [600, 128]
        xh, xl = _limbs(x)
        sh, sl = _limbs(-(x * x))

        idx = lo[:, None] + np.arange(L)[None, :]
        valid = np.arange(L)[None, :] < ln[:, None]
        idxc = np.clip(idx, 0, K - 1)
        y = S * (cts[idxc] - a[:, None])                        # [600, L]
        yh, yl = _limbs(y)
        w2yh = np.where(valid, 2.0 * yh, 0.0)
        w2yl = np.where(valid, 2.0 * yl, 0.0)
        qbv = np.where(valid, -((yh + yl) ** 2), -FAR).astype(np.float32)

        for c in range(NCORES):
            blks = np.arange(c * BPB, (c + 1) * BPB)
            for s_i, gblk in enumerate(blks):
                g = b * NGH + s_i // GBLK
                gb = s_i % GBLK
                r = KR * gb
                # moving rows (t-side)
                wm_all[c, r + 0, 1, g, :] = sh[gblk]
                wm_all[c, r + 1, 1, g, :] = sl[gblk]
                wm_all[c, r + 2, 1, g, :] = xh[gblk]
                wm_all[c, r + 3, 1, g, :] = xh[gblk]
                wm_all[c, r + 4, 1, g, :] = xl[gblk]
                # stationary cols (c-side), block diagonal at cols 8*gb+j
                cols = slice(L * gb, L * gb + L)
                wm_all[c, r + 0, 0, g, cols] = 1.0
                wm_all[c, r + 1, 0, g, cols] = 1.0
                wm_all[c, r + 2, 0, g, cols] = w2yh[gblk]
                wm_all[c, r + 3, 0, g, cols] = w2yl[gblk]
                wm_all[c, r + 4, 0, g, cols] = w2yh[gblk]
                qb_all[c, cols, g] = qbv[gblk]
                meta[c]["lo"][g, gb] = lo[gblk]
                meta[c]["ln"][g, gb] = ln[gblk]

    in_maps = [
        {
            "wm": np.ascontiguousarray(wm_all[c]),
            "qb": np.ascontiguousarray(qb_all[c]),
        }
        for c in range(NCORES)
    ]
    return in_maps, meta, cts_sorted


def _combine(results, meta):
    d2_tot = np.zeros(B, np.float64)
    gmax = np.full((B, K), -np.inf)
    for c, res in enumerate(results):
        d2 = np.asarray(res["d2"], np.float64)               # [128, B]
        d2_tot += d2.sum(axis=0)
        d1 = np.asarray(res["d1"], np.float64)               # [128, NG, 1]
        lo, ln = meta[c]["lo"], meta[c]["ln"]
        for g in range(NG):
            h = g // NGH
            for gb in range(GBLK):
                ll = ln[g, gb]
                if ll == 0:
                    continue
                li = lo[g, gb]
                vals = d1[L * gb : L * gb + ll, g, 0]
                np.maximum.at(gmax[h], np.arange(li, li + ll), vals)
    assert np.isfinite(gmax).all(), "uncovered center in dir1 combine"
    dir2 = -d2_tot / (S * S)
    dir1 = (-gmax / (S * S)).sum(axis=1)
    return np.float32((dir1 + dir2).mean())


def kernel(target: np.ndarray, bin_edges: np.ndarray) -> np.ndarray:
    in_maps, meta, _ = _prep(target, bin_edges)
    nc = _get_compiled()
    res = run_bass_kernel_spmd(nc, in_maps, list(range(NCORES))).results
    out = _combine(res, meta)
    return np.asarray(out, dtype=np.float32)


# revision 22
# speedup vs baseline: 4.0869x; 1.1176x over previous
"""Chamfer distance v14: block-sparse KNN, bf16-limb grouped matmuls, no ScalarE.

Host sorts points per batch; 128-point sorted blocks each have a contiguous
run of <=8 candidate sorted centers (data max 6). 16 blocks pack into one
bf16 matmul via a block-diagonal stationary; slot j of block gb sits at
output partition 8*gb+j. Grid value (negated scaled squared distance):
    G = sh + sl + xh*2yh + xh*2yl + xl*2yh + bias(-y^2)
with x = S(t - a_blk) split into bf16 limbs (xh, xl), s = -x^2 limbs
(sh, sl), y = S(c - a_blk) limbs (yh, yl); products are exact in bf16 x
bf16 -> f32. K = 5 rows per block * 16 blocks = 80. The -y^2 term rides the
per-partition scalar of the DVE squash, so ScalarE is never used (saves the
1.3us ACT_TABLE_LOAD and frees its DMA queue).

Per group g (PSUM [128 blockslots, 128 points], 4 PSUM banks ping-ponged):
  squash: DVE tensor_scalar add(-y^2) -> f16 SBUF
  dir1 (per-center min over points): DVE free-axis max-folds -> [128, NG]
  dir2 (per-point min over centers): batched DMA-transpose, DVE max-tree over
        own block's 8 slots, DVE reduce-add per batch.
Host: sorting, block metadata (searchsorted of block bounds), final combines.
"""

import sys

if "/opt/trn_rl_repo" not in sys.path:
    sys.path.insert(0, "/opt/trn_rl_repo")

import numpy as np
import ml_dtypes

import concourse.bass as bass
import concourse.tile as tile
from concourse import bacc, mybir
from concourse.bass_utils import run_bass_kernel_spmd

B = 2
N = 76800
E = 257
K = 256
NCORES = 8
BLK = 128
NBLK_B = N // BLK          # 600 blocks per batch
BPB = NBLK_B // NCORES     # 75 blocks per (core, batch)
GBLK = 16                  # blocks per matmul group
NGH = 5                    # groups per batch-half (5*16 = 80 slots, 75 real)
NG = B * NGH               # 10 groups per core
L = 8                      # candidate-center slots per block (data max is 6)
KR = 5                     # limb rows per block
KK = KR * GBLK             # 80 contraction rows per group matmul
S = 1024.0
FAR = 3.0e4

F32 = mybir.dt.float32
F16 = mybir.dt.float16
BF16 = mybir.dt.bfloat16
MAX = mybir.AluOpType.max
ADD = mybir.AluOpType.add
AX = mybir.AxisListType
BF = ml_dtypes.bfloat16


def _build_kernel(nc, tc, wm_in, qb_in, d1_out):
    from contextlib import ExitStack

    ctx = ExitStack()
    sb = ctx.enter_context(tc.tile_pool(name="sb", bufs=1))
    psum_pool = ctx.enter_context(tc.tile_pool(name="ps", bufs=1, space="PSUM"))

    wm_sb = sb.tile([KK, NG, 2, 128], BF16, tag="wm")
    qb_sb = sb.tile([128, NG], F32, tag="qb")
    # one full PSUM bank per tile so dependency tracking stays per-group
    psb = [
        psum_pool.tile([128, 512], F32, tag=f"ps{i}", name=f"ps{i}")
        for i in range(4)
    ]
    gsb = sb.tile([128, NG, 128], F16, tag="gsb")
    tt = sb.tile([128, NG, 128], F16, tag="tt")
    l1 = sb.tile([128, NG, GBLK, 4], F16, tag="l1")
    l2 = sb.tile([128, NG, GBLK, 2], F16, tag="l2")
    l3 = sb.tile([128, NG, GBLK, 1], F16, tag="l3")
    f1 = sb.tile([128, NG, 64], F16, tag="f1")
    f2 = sb.tile([128, NG, 32], F16, tag="f2")
    f3 = sb.tile([128, NG, 16], F16, tag="f3")
    f4 = sb.tile([128, NG, 8], F16, tag="f4")
    outb = sb.tile([128, NG + B], F32, tag="outb")

    ttv = tt[:].rearrange("p g (c j) -> p g c j", j=L)

    # stream inputs in 2-group chunks, alternating HWDGE queues
    nc.scalar.dma_start(qb_sb[:], qb_in)
    for i, g0 in enumerate(range(0, NG, 2)):
        gs = slice(g0, g0 + 2)
        eng = nc.sync if i % 2 == 0 else nc.scalar
        eng.dma_start(wm_sb[:, gs], wm_in[:, gs])

    def mm(g):
        ps = psb[g % 4]
        nc.tensor.matmul(
            ps[:, 0:128], wm_sb[:, g, 0], wm_sb[:, g, 1], start=True, stop=True
        )
        return ps

    def squash(g, ps):
        nc.vector.tensor_scalar(
            gsb[:, g], ps[:, 0:128], qb_sb[:, g : g + 1], None, op0=ADD
        )

    def fchain(h):
        hs = slice(h * NGH, (h + 1) * NGH)
        nc.vector.tensor_tensor(
            f1[:, hs], gsb[:, hs, 0:64], gsb[:, hs, 64:128], op=MAX
        )
        nc.vector.tensor_tensor(f2[:, hs], f1[:, hs, 0:32], f1[:, hs, 32:64], op=MAX)
        nc.vector.tensor_tensor(f3[:, hs], f2[:, hs, 0:16], f2[:, hs, 16:32], op=MAX)
        nc.vector.tensor_tensor(f4[:, hs], f3[:, hs, 0:8], f3[:, hs, 8:16], op=MAX)
        nc.vector.tensor_reduce(
            out=outb[:, hs].rearrange("p (g o) -> p g o", o=1),
            in_=f4[:, hs], op=MAX, axis=AX.X,
        )

    def ltree(h):
        hs = slice(h * NGH, (h + 1) * NGH)
        nc.vector.tensor_tensor(
            l1[:, hs], ttv[:, hs, :, 0:4], ttv[:, hs, :, 4:8], op=MAX
        )
        nc.vector.tensor_tensor(l2[:, hs], l1[:, hs, :, 0:2], l1[:, hs, :, 2:4], op=MAX)
        nc.vector.tensor_tensor(l3[:, hs], l2[:, hs, :, 0:1], l2[:, hs, :, 1:2], op=MAX)
        nc.vector.tensor_reduce(
            out=outb[:, NG + h : NG + h + 1], in_=l3[:, hs, :, 0], op=ADD,
            axis=AX.XY,
        )

    # pipeline: PE runs ahead on the 4 psum banks; DVE squashes trail.
    # Program-order constraint: squash(g) must be emitted BEFORE mm(g+4)
    # reuses its bank. Per-half tails overlap the other half's matmuls.
    pss = {}
    for g in range(4):
        pss[g] = mm(g)
    for g in range(6):
        squash(g, pss[g])
        pss[g + 4] = mm(g + 4)
    nc.sync.dma_start_transpose(tt[:, 0:NGH], gsb[:, 0:NGH])
    fchain(0)
    for g in range(6, 9):
        squash(g, pss[g])
    nc.scalar.dma_start_transpose(tt[:, NGH : NGH + 4], gsb[:, NGH : NGH + 4])
    ltree(0)
    squash(9, pss[9])
    nc.sync.dma_start_transpose(tt[:, 9:10], gsb[:, 9:10])
    fchain(1)
    ltree(1)

    nc.scalar.dma_start(d1_out, outb[:])
    ctx.close()


_CACHE = {}


def _get_compiled():
    if "nc" in _CACHE:
        return _CACHE["nc"]
    nc = bacc.Bacc(
        "TRN2",
        target_bir_lowering=False,
        debug=False,
        enable_asserts=False,
        num_devices=NCORES,
    )
    wm_in = nc.dram_tensor("wm", [KK, NG, 2, 128], BF16, kind="ExternalInput").ap()
    qb_in = nc.dram_tensor("qb", [128, NG], F32, kind="ExternalInput").ap()
    d1_out = nc.dram_tensor("d1", [128, NG + B], F32, kind="ExternalOutput").ap()

    with tile.TileContext(nc) as tc:
        _build_kernel(nc, tc, wm_in, qb_in, d1_out)
    nc.compile()
    _CACHE["nc"] = nc
    return nc


def _limbs(v):
    hi = v.astype(BF).astype(np.float64)
    lo = (v - hi).astype(BF).astype(np.float64)
    return hi, lo


def _prep(target: np.ndarray, bin_edges: np.ndarray):
    """Host prep: sort, block metadata, packed bf16 W/M rows + f32 bias."""
    target = np.asarray(target, dtype=np.float32).reshape(B, N)
    edges = np.asarray(bin_edges, dtype=np.float64)

    wm_all = np.zeros((NCORES, KK, NG, 2, 128), BF)
    qb_all = np.zeros((NCORES, 128, NG), np.float32)
    meta = [
        {"lo": np.zeros((NG, GBLK), np.int64), "ln": np.zeros((NG, GBLK), np.int64)}
        for _ in range(NCORES)
    ]
    cts_sorted = []

    for b in range(B):
        pts = np.sort(target[b])
        cts = np.sort(0.5 * (edges[b, :-1] + edges[b, 1:]))
        cts_sorted.append(cts)
        pts64 = pts.astype(np.float64)

        t0s = pts64[0::BLK]
        t1s = pts64[BLK - 1 :: BLK]
        tprev = np.concatenate(([-np.inf], t1s[:-1]))
        tnext = np.concatenate((t0s[1:], [np.inf]))

        lo = np.minimum(
            np.searchsorted(cts, tprev, side="right"),
            np.searchsorted(cts, t0s, side="right") - 1,
        )
        lo = np.maximum(lo, 0)
        hi = np.maximum(
            np.searchsorted(cts, tnext, side="left") - 1,
            np.searchsorted(cts, t1s, side="left"),
        )
        hi = np.minimum(hi, K - 1)
        ln = hi - lo + 1
        assert ln.max() <= L, f"candidate run {ln.max()} exceeds L={L}"

        a = t0s
        x = S * (pts64.reshape(NBLK_B, BLK) - a[:, None])       # [600, 128]
        xh, xl = _limbs(x)
        sh, sl = _limbs(-(x * x))

        idx = lo[:, None] + np.arange(L)[None, :]
        valid = np.arange(L)[None, :] < ln[:, None]
        idxc = np.clip(idx, 0, K - 1)
        y = S * (cts[idxc] - a[:, None])                        # [600, L]
        yh, yl = _limbs(y)
        w2yh = np.where(valid, 2.0 * yh, 0.0)
        w2yl = np.where(valid, 2.0 * yl, 0.0)
        qbv = np.where(valid, -((yh + yl) ** 2), -FAR).astype(np.float32)

        for c in range(NCORES):
            blks = np.arange(c * BPB, (c + 1) * BPB)
            for s_i, gblk in enumerate(blks):
                g = b * NGH + s_i // GBLK
                gb = s_i % GBLK
                r = KR * gb
                # moving rows (t-side)
                wm_all[c, r + 0, g, 1, :] = sh[gblk]
                wm_all[c, r + 1, g, 1, :] = sl[gblk]
                wm_all[c, r + 2, g, 1, :] = xh[gblk]
                wm_all[c, r + 3, g, 1, :] = xh[gblk]
                wm_all[c, r + 4, g, 1, :] = xl[gblk]
                # stationary cols (c-side), block diagonal at cols 8*gb+j
                cols = slice(L * gb, L * gb + L)
                wm_all[c, r + 0, g, 0, cols] = 1.0
                wm_all[c, r + 1, g, 0, cols] = 1.0
                wm_all[c, r + 2, g, 0, cols] = w2yh[gblk]
                wm_all[c, r + 3, g, 0, cols] = w2yl[gblk]
                wm_all[c, r + 4, g, 0, cols] = w2yh[gblk]
                qb_all[c, cols, g] = qbv[gblk]
                meta[c]["lo"][g, gb] = lo[gblk]
                meta[c]["ln"][g, gb] = ln[gblk]

    in_maps = [
        {
            "wm": np.ascontiguousarray(wm_all[c]),
            "qb": np.ascontiguousarray(qb_all[c]),
        }
        for c in range(NCORES)
    ]
    return in_maps, meta, cts_sorted


def _combine(results, meta):
    d2_tot = np.zeros(B, np.float64)
    gmax = np.full((B, K), -np.inf)
    for c, res in enumerate(results):
        out = np.asarray(res["d1"], np.float64)              # [128, NG + B]
        d2_tot += out[:, NG:].sum(axis=0)
        d1 = out[:, :NG, None]
        lo, ln = meta[c]["lo"], meta[c]["ln"]
        for g in range(NG):
            h = g // NGH
            for gb in range(GBLK):
                ll = ln[g, gb]
                if ll == 0:
                    continue
                li = lo[g, gb]
                vals = d1[L * gb : L * gb + ll, g, 0]
                np.maximum.at(gmax[h], np.arange(li, li + ll), vals)
    assert np.isfinite(gmax).all(), "uncovered center in dir1 combine"
    dir2 = -d2_tot / (S * S)
    dir1 = (-gmax / (S * S)).sum(axis=1)
    return np.float32((dir1 + dir2).mean())


def kernel(target: np.ndarray, bin_edges: np.ndarray) -> np.ndarray:
    in_maps, meta, _ = _prep(target, bin_edges)
    nc = _get_compiled()
    res = run_bass_kernel_spmd(nc, in_maps, list(range(NCORES))).results
    out = _combine(res, meta)
    return np.asarray(out, dtype=np.float32)
